# revision 3
# baseline (speedup 1.0000x reference)
"""Trainium2 Bass kernel for nn_MessageFunction (GNN message passing).

msg[b,o,n] = sum_d We[o,d]*e_vw[b,d,n] + sum_d Ww[o,d]*h_w[b,d,n] + (be+bw)[o]
B=128, D=768, N=256; data-parallel over B across 8 cores (16 batches/core).

Design (hardware-measured on trn2):
- Inputs host-packed partition-major [NBLK, 128, KT, NCOL] fp16: each block
  load walks HBM linearly (6 KB per partition, 768 KB sequential per block)
  instead of 1 KB runs at 4 KB stride.
- Weights host-packed m-major [MT, 128, KT, 128] fp16 so each per-m prefill
  chunk is one linear 196 KB stream; bias prepacked [128, MT] f32.
- Output written [NBLK, MT, 128, NCOL] fp16 in instruction order (each
  (c,m) store is one linear 128 KB stream); host unpacks/upcasts to
  [BPC, D, N] f32. Halves output HBM traffic vs f32.
- rings: e+weights/sync, h/scalar, out/gpsimd.
"""
import numpy as np
import concourse.tile as tile
from concourse import bacc, mybir
from concourse.bass_utils import run_bass_kernel_spmd

try:  # persistent XLA cache: repeated fresh-process runs skip the NEFF compile
    import jax
    jax.config.update("jax_compilation_cache_dir", "/tmp/.jax_kernel_cache")
    jax.config.update("jax_persistent_cache_min_compile_time_secs", 0.5)
except Exception:
    pass

B, D, NN = 128, 768, 256
NCORES = 8
BPC = B // NCORES          # 16 batches per core
PAIR = 2                   # batches per 512-wide moving block
NBLK = BPC // PAIR         # 8 column blocks per pass
NCOL = PAIR * NN           # 512 moving columns
KT = D // 128              # 6 contraction tiles per input matrix
MT = D // 128              # 6 output row tiles
F32 = mybir.dt.float32
F16 = mybir.dt.float16
DT = F16
NPDT = np.float16


def build(repeat: int = 1, loop_repeat: int = 1):
    nc = bacc.Bacc("TRN2", target_bir_lowering=False, debug=False,
                   num_devices=NCORES)
    e = nc.dram_tensor("e", [NBLK, 128, KT, NCOL], DT, kind="ExternalInput").ap()
    h = nc.dram_tensor("h", [NBLK, 128, KT, NCOL], DT, kind="ExternalInput").ap()
    wep = nc.dram_tensor("wep", [MT, 128, KT, 128], DT, kind="ExternalInput").ap()
    wwp = nc.dram_tensor("wwp", [MT, 128, KT, 128], DT, kind="ExternalInput").ap()
    biasp = nc.dram_tensor("biasp", [128, MT], F32, kind="ExternalInput").ap()
    out = nc.dram_tensor("out", [NBLK, MT, 128, NCOL], F16,
                         kind="ExternalOutput").ap()

    with tile.TileContext(nc) as tc:
        with (
            tc.tile_pool(name="wpool", bufs=1) as wpool,
            tc.tile_pool(name="xpool", bufs=3) as xpool,
            tc.tile_pool(name="opool", bufs=6) as opool,
            tc.tile_pool(name="pspool", bufs=8, space="PSUM") as pspool,
        ):
            we_t = wpool.tile([128, MT, KT, 128], DT)
            ww_t = wpool.tile([128, MT, KT, 128], DT)
            bias_t = wpool.tile([128, MT], F32)
            # need-order: bias + m=0 weights first, rest behind the first
            # column block's loads (HWDGE executes FIFO per engine ring).
            nc.sync.dma_start(bias_t[:], biasp)
            nc.sync.dma_start(we_t[:, 0], wep[0])
            nc.sync.dma_start(ww_t[:, 0], wwp[0])

            first = [True]

            def _block(c):
                et = xpool.tile([128, KT, NCOL], DT, tag="et", name="et")
                ht = xpool.tile([128, KT, NCOL], DT, tag="ht", name="ht")
                if first[0]:
                    # per-k loads so the first matmul group starts after
                    # ~0.4MB of DMA; Tile's subtile deps gate MM k on its
                    # own slice only.
                    first[0] = False
                    for k in range(KT):
                        nc.sync.dma_start(et[:, k], e[c, :, k])
                        nc.scalar.dma_start(ht[:, k], h[c, :, k])
                    for m in range(1, MT):
                        nc.sync.dma_start(we_t[:, m], wep[m])
                        nc.sync.dma_start(ww_t[:, m], wwp[m])
                else:
                    nc.sync.dma_start(et[:], e[c])
                    nc.scalar.dma_start(ht[:], h[c])
                for m in range(MT):
                    ps = pspool.tile([128, NCOL], F32, name="ps")
                    for k in range(KT):
                        nc.tensor.matmul(ps[:], we_t[:, m, k], et[:, k],
                                         start=(k == 0), stop=False)
                    for k in range(KT):
                        nc.tensor.matmul(ps[:], ww_t[:, m, k], ht[:, k],
                                         start=False, stop=(k == KT - 1))
                    res = opool.tile([128, NCOL], F16, name="res")
                    nc.scalar.activation(
                        res[:], ps[:], mybir.ActivationFunctionType.Identity,
                        bias=bias_t[:, m:m + 1], scale=1.0)
                    nc.gpsimd.dma_start(out[c, m], res[:])

            def body():
                for _ in range(repeat):
                    for c in range(NBLK):
                        _block(c)

            if loop_repeat > 1:
                with tc.For_i(0, loop_repeat, 1,
                              hint_engines=(mybir.EngineType.PE,)):
                    body()
            else:
                body()
    nc.compile()
    return nc


def _prep_in_maps(h_w, e_vw, We, be, Ww, bw):
    e_vw = np.asarray(e_vw, dtype=np.float32).astype(NPDT)
    h_w = np.asarray(h_w, dtype=np.float32).astype(NPDT)

    def wpack(W):
        # W[o,d] -> wp[m,p,k,q] = W[m*128+q, k*128+p]  (lhsT = W.T tiles)
        wT = np.asarray(W, dtype=np.float32).T.astype(NPDT)  # [d, o]
        return np.ascontiguousarray(
            wT.reshape(KT, 128, MT, 128).transpose(2, 1, 0, 3))

    bias = (np.asarray(be, dtype=np.float32)
            + np.asarray(bw, dtype=np.float32)).astype(np.float32)
    biasp = np.ascontiguousarray(bias.reshape(MT, 128).T)   # [128, MT]

    def slab(x, c):
        # [BPC,D,NN] -> [NBLK,128,KT,NCOL]: s[c,p,k,j*NN+n] = x[c*PAIR+j, k*128+p, n]
        s = x[c * BPC:(c + 1) * BPC].reshape(NBLK, PAIR, KT, 128, NN)
        return np.ascontiguousarray(
            s.transpose(0, 3, 2, 1, 4).reshape(NBLK, 128, KT, NCOL))

    wep, wwp = wpack(We), wpack(Ww)
    return [
        {"e": slab(e_vw, c), "h": slab(h_w, c),
         "wep": wep, "wwp": wwp, "biasp": biasp}
        for c in range(NCORES)
    ]


def _unpack_out(res):
    # [NBLK,MT,128,NCOL] f16 -> [BPC,D,NN] f32
    s = res.reshape(NBLK, MT, 128, PAIR, NN).astype(np.float32)
    return s.transpose(0, 3, 1, 2, 4).reshape(BPC, D, NN)


_NC_CACHE = []


def kernel(h_v, h_w, e_vw, We, be, Ww, bw):
    if not _NC_CACHE:
        _NC_CACHE.append(build())
    nc = _NC_CACHE[0]
    in_maps = _prep_in_maps(h_w, e_vw, We, be, Ww, bw)
    r = run_bass_kernel_spmd(nc, in_maps, core_ids=list(range(NCORES)))
    return np.concatenate(
        [_unpack_out(r.results[c]["out"]) for c in range(NCORES)], axis=0)
